# revision 18
# baseline (speedup 1.0000x reference)
"""SAGAN-style self-attention on 8 trn2 cores: data-parallel over batch.

Per core (one batch image): x^T [256,4096] bf16 in, out [4096,256] f32 out.

v3 (on top of v2's fp8-DoubleRow attention):
  - input DMAs spread across the SP and ACT hardware DGE queues, x chunks
    first, so the Q/K projection starts ~20us earlier
  - Q and K projections fused into one [64,512] matmul per x chunk
    (stationary = [Wf|Wg]), halving projection matmul count
  - qt_rep strip replication and kt_stack regrouping done per 512-column
    chunk so attention tile 0 is unblocked early
  - combined out-bias (bc) computed after the projections (it heads the
    PE program order otherwise and stalls everything on its DMAs)
  - out-projection in fp8 DoubleRow too: osb = fp8(O'/64), Wo8 = fp8(64*Wo)
    (scales cancel), one matmul per query chunk

  Attention per 512-query tile, per group of 4 key blocks (= 2 fp8 pairs):
    T    = KT_blk.T @ QT_tile   4x row-packed bf16 [128,2048] PSUM
    expT = exp(T)               ACT, PSUM->SBUF, fp8e4 out, one instr
    O'  += V_pk.T @ expT        fp8 DoubleRow, [128,512] f32 PSUM x2
    Z   += (16*ones).T @ expT   fp8 DoubleRow, [16,512] (x16 cancels V's)
  tail: osb = fp8(O'/64); Z -> SBUF -> 4 PE transposes -> [128,4];
  zr = 1/Z; per 128-query chunk: f = osb^T @ Wo8 (DR), out = f*zr + bc_rep
  (one DVE scalar_tensor_tensor); DMA out rows.
"""

import sys

if "/opt/trn_rl_repo" not in sys.path:
    sys.path.insert(0, "/opt/trn_rl_repo")

import ml_dtypes
import numpy as np

import concourse.bass as bass
import concourse.mybir as mybir
import concourse.tile as tile
from concourse.bass_utils import run_bass_kernel_spmd

B, H, W, C = 8, 64, 64, 256
KEY = 32
N = H * W          # 4096 tokens
NT = 512           # query tile (free dim per matmul)
NTILES = N // NT   # 8
MB = 128           # key block (contraction chunk)
NMB = N // MB      # 32
NPB = NMB // 2     # 16 fp8 pair-blocks
GRP = 4            # key blocks per group: one per PE row strip
NGRP = NMB // GRP  # 8 groups per query tile
NQC = NT // 128    # 4 query chunks per tile (out-proj granularity)

BF16 = mybir.dt.bfloat16
F32 = mybir.dt.float32
F8 = mybir.dt.float8e4
FT = mybir.ActivationFunctionType
DR = mybir.MatmulPerfMode.DoubleRow
VSCALE = 16.0      # fp8 V pre-scale; cancelled via the Z ones value


def build_nc() -> bass.Bass:
    nc = bass.Bass()

    xT = nc.declare_dram_parameter("xT", [2, 128, N], BF16, isOutput=False)
    wfg = nc.declare_dram_parameter("wfg", [2, 128, 2 * KEY], BF16, isOutput=False)
    wh = nc.declare_dram_parameter("wh", [2, 128, C], BF16, isOutput=False)
    bfgc = nc.declare_dram_parameter("bfgc", [2 * KEY, 1], F32, isOutput=False)
    bhc = nc.declare_dram_parameter("bhc", [2, 128, 1], BF16, isOutput=False)
    wobh = nc.declare_dram_parameter("wobh", [2, 128, C], BF16, isOutput=False)
    bor = nc.declare_dram_parameter("bor", [1, C], BF16, isOutput=False)
    outN = nc.declare_dram_parameter("outN", [N, C], F32, isOutput=True)

    with tile.TileContext(nc) as tc:
        with (
            tc.tile_pool(name="const", bufs=1) as const,
            tc.tile_pool(name="xp", bufs=1) as xp,
            tc.tile_pool(name="vp", bufs=1) as vp,
            tc.tile_pool(name="qk", bufs=1) as qk,
            tc.tile_pool(name="ep", bufs=3) as ep,
            tc.tile_pool(name="osb", bufs=2) as osbp,
            tc.tile_pool(name="zp", bufs=2) as zp,
            tc.tile_pool(name="outp", bufs=4) as outp,
            # PSUM: 8 banks. pt "t" [128,2048]f32 = 4; po o0/o1 = 2;
            # pz "z" ([16,512] then [128,4] transposed) = 1, "f" = 1.
            tc.tile_pool(name="pt", bufs=1, space="PSUM") as pt,
            tc.tile_pool(name="po", bufs=1, space="PSUM") as po,
            tc.tile_pool(name="pz", bufs=1, space="PSUM") as pz,
        ):
            # ---- constants (no DMA deps) ----
            ones16 = const.tile([128, 2, 16], F8)   # Z-matmul lhsT (=VSCALE)
            ones_r = const.tile([1, 1], BF16)       # bc preload rhs
            id1 = const.tile([1, 1], F32)           # PE-transpose identity
            ones_m32 = const.tile([1, 128], F32)
            nc.vector.memset(ones16, VSCALE)
            nc.vector.memset(ones_r, 1.0)
            nc.vector.memset(id1, 1.0)
            nc.vector.memset(ones_m32, 1.0)

            # ---- input DMAs: x chunks + QK weights first, spread across
            # the two hardware DGE queues (SP + ACT) ----
            wfg_sb = const.tile([128, 2, 2 * KEY], BF16)
            wh_sb = const.tile([128, 2, C], BF16)
            wobh_sb = const.tile([128, 2, C], BF16)
            bfg_sb = const.tile([2 * KEY, 1], F32)
            bh_sb = const.tile([128, 2, 1], BF16)
            bo_sb = const.tile([1, C], BF16)

            xts = [
                [xp.tile([128, NT], BF16, name=f"xt{cc}_{h}") for h in range(NTILES)]
                for cc in range(2)
            ]
            qs = [nc.sync, nc.scalar]
            for cc in range(2):
                qs[cc].dma_start(out=wfg_sb[:, cc, :], in_=wfg[cc])
            nc.sync.dma_start(out=bfg_sb, in_=bfgc[:])
            for h in range(NTILES):
                for cc in range(2):
                    qs[(h + cc) % 2].dma_start(
                        out=xts[cc][h], in_=xT[cc, :, h * NT:(h + 1) * NT]
                    )
            for cc in range(2):
                qs[cc].dma_start(out=wh_sb[:, cc, :], in_=wh[cc])
            for cc in range(2):
                qs[cc].dma_start(out=wobh_sb[:, cc, :], in_=wobh[cc])
            nc.sync.dma_start(out=bh_sb, in_=bhc[:])
            nc.scalar.dma_start(out=bo_sb, in_=bor[:])

            def xs(cc, start, width):
                # column slice of xT chunk cc; never crosses a tile boundary
                h = start // NT
                assert (start + width - 1) // NT == h
                return xts[cc][h][:, start - h * NT: start - h * NT + width]

            pp_i = 0

            def proj_psum(shape):
                # alternate between two borrowed slots for double-buffering
                nonlocal pp_i
                pp_i += 1
                if pp_i % 2:
                    return pt.tile(shape, F32, tag="t", name=f"projps{pp_i}")
                return pz.tile(shape, F32, tag="f", name=f"projps{pp_i}")

            # ---- fused Q/K projection: one [64, 512] matmul per chunk ----
            # qkt rows 0-31 = Q^T, rows 32-63 = K^T (partition-aligned with
            # the psum so the DVE bias-adds never cross partitions; the
            # strip replication below is DMA, which can move partitions)
            qkt = qk.tile([2 * KEY, N], BF16)
            qt_rep = qk.tile([128, N], BF16)
            kt_stack = qk.tile([128, NMB // 4, MB], BF16)
            for g in range(NTILES):
                sl = slice(g * NT, (g + 1) * NT)
                ps = proj_psum([2 * KEY, NT])
                for cc in range(2):
                    nc.tensor.matmul(
                        ps, wfg_sb[:, cc, :], xs(cc, g * NT, NT),
                        start=(cc == 0), stop=(cc == 1),
                    )
                nc.vector.tensor_scalar_add(qkt[:, sl], ps, bfg_sb)
                # replicate Q strips + regroup K for this chunk immediately
                for i in range(4):
                    qs[i % 2].dma_start(
                        out=qt_rep[32 * i:32 * (i + 1), sl], in_=qkt[0:KEY, sl]
                    )
                for i in range(4):
                    b = 4 * g + i
                    qs[(i + 1) % 2].dma_start(
                        out=kt_stack[32 * i:32 * (i + 1), g, :],
                        in_=qkt[KEY:2 * KEY, b * MB:(b + 1) * MB],
                    )

            # ---- V projection -> 16 fp8 pair-tiles [128, 2, 256] (x16) ----
            v_pk = [
                vp.tile([128, 2, C], F8, tag=f"v{p}", name=f"v{p}")
                for p in range(NPB)
            ]
            for mb in range(NMB):
                ps = proj_psum([128, C])
                for cc in range(2):
                    nc.tensor.matmul(
                        ps, xs(cc, mb * MB, MB), wh_sb[:, cc, :],
                        start=(cc == 0), stop=(cc == 1),
                    )
                nc.vector.tensor_scalar_mul(v_pk[mb // 2][:, mb % 2, :], ps, VSCALE)

            # ---- combined out-bias bc = Wo^T bh + bo, replicated to all
            # query partitions: bc_rep [128, 256] f32 (emitted after the
            # projections so its DMA waits don't stall the PE stream) ----
            bcr_ps = pz.tile([1, C], F32, tag="f", name="bcrow")
            nc.tensor.matmul(bcr_ps, ones_r, bo_sb, start=True, stop=False)
            for cc in range(2):
                nc.tensor.matmul(
                    bcr_ps, bh_sb[:, cc, :], wobh_sb[:, cc, :],
                    start=False, stop=(cc == 1),
                )
            bc_row = const.tile([1, C], F32)
            nc.vector.tensor_copy(out=bc_row, in_=bcr_ps)
            bcr2_ps = pz.tile([128, C], F32, tag="f", name="bcrep")
            nc.tensor.matmul(bcr2_ps, ones_m32, bc_row, start=True, stop=True)
            bc_rep = const.tile([128, C], F32)
            nc.vector.tensor_copy(out=bc_rep, in_=bcr2_ps)

            # ---- attention: flat software pipeline over (query-tile, group) ----
            cur = {}           # nt -> (o_ps pair, z_ps)

            def emit_tail(nt):
                """Tail for tile nt, as a list of pieces to spread across the
                next tile's group slots (each group has ~0.7us of PE slack;
                dumping the whole tail at once stalls the T/exp pipeline)."""
                o_ps, z_ps = cur.pop(nt)
                pieces = []

                def p_osb_z():
                    # osb: bf16 casts of unnormalized O'; Z row to SBUF
                    osb = [osbp.tile([128, NT], BF16, tag=f"os{cc}",
                                     name=f"os{cc}_{nt}") for cc in range(2)]
                    for cc in range(2):
                        nc.vector.tensor_copy(out=osb[cc], in_=o_ps[cc])
                    zsb = zp.tile([1, NT], F32, tag="zs", name=f"zs{nt}")
                    nc.vector.tensor_copy(out=zsb, in_=z_ps[0:1, :])
                    st["osb"], st["zsb"] = osb, zsb

                def p_zr():
                    # Z -> 4 PE transposes -> [128,4] -> zr = 1/Z.  zt and f
                    # share the "f" bank, allocated here in access order (the
                    # "z" tag must stay clean: the next tile's Z accumulator
                    # is already live there).
                    zt_ps = pz.tile([128, NQC], F32, tag="f", name=f"zt{nt}")
                    for qc in range(NQC):
                        nc.tensor.transpose(
                            zt_ps[:, qc:qc + 1],
                            st["zsb"][:, qc * 128:(qc + 1) * 128], id1,
                        )
                    zr = zp.tile([128, NQC], F32, tag="zr", name=f"zr{nt}")
                    nc.vector.reciprocal(out=zr, in_=zt_ps)
                    st["zr"] = zr
                    st["f"] = pz.tile([128, 2, C], F32, tag="f", name=f"f{nt}")

                def mk_qc(qc):
                    def p_qc():
                        # out[q, c'] = (osb_chunk^T @ Wo) * zr + bc_rep
                        qsl = slice(qc * 128, (qc + 1) * 128)
                        fp = st["f"][:, qc % 2, :]
                        for cc in range(2):
                            nc.tensor.matmul(
                                fp, st["osb"][cc][:, qsl], wobh_sb[:, cc, :],
                                start=(cc == 0), stop=(cc == 1),
                            )
                        out_sb = outp.tile([128, C], F32, tag="out",
                                           name=f"o{nt}_{qc}")
                        nc.vector.scalar_tensor_tensor(
                            out=out_sb, in0=fp, scalar=st["zr"][:, qc:qc + 1],
                            in1=bc_rep,
                            op0=mybir.AluOpType.mult, op1=mybir.AluOpType.add,
                        )
                        nc.sync.dma_start(
                            out=outN[nt * NT + qc * 128:
                                     nt * NT + (qc + 1) * 128, :],
                            in_=out_sb,
                        )
                    return p_qc

                st = {}
                pieces.append(p_osb_z)
                pieces.append(p_zr)
                for qc in range(NQC):
                    pieces.append(mk_qc(qc))
                return pieces

            def emit_oz(nt, g, e_grp):
                """O'/Z accumulation for group g of tile nt (+tail after last)"""
                if g == 0:
                    cur[nt] = (
                        [po.tile([128, NT], F32, tag="o0", name=f"o0_{nt}"),
                         po.tile([128, NT], F32, tag="o1", name=f"o1_{nt}")],
                        pz.tile([16, NT], F32, tag="z", name=f"z{nt}"),
                    )
                o_ps, z_ps = cur[nt]
                for pi in range(2):
                    pb = 2 * g + pi
                    first, last = pb == 0, pb == NPB - 1
                    e_pk = e_grp[:, pi]  # [128, 2, 512]
                    for cc in range(2):
                        nc.tensor.matmul(
                            o_ps[cc],
                            v_pk[pb][:, :, cc * 128:(cc + 1) * 128],
                            e_pk,
                            start=first, stop=last, perf_mode=DR,
                        )
                    nc.tensor.matmul(
                        z_ps, ones16, e_pk, start=first, stop=last, perf_mode=DR,
                    )
                if g == NGRP - 1:
                    return emit_tail(nt)
                return []

            # Pipelined one group deep: T-pack(i) ... O'/Z(i-1). exp(i) on ACT
            # hides under O'/Z(i-1) on PE; the per-tile tail hides under the
            # next tile's first T-pack/O'Z groups.
            prev = None
            pending = []       # tail pieces awaiting a group slot
            for nt in range(NTILES):
                nsl = slice(nt * NT, (nt + 1) * NT)
                for g in range(NGRP):
                    # 4-way row-packed score matmuls: strip j contracts its own
                    # 32 rows of the PE array concurrently.
                    t_ps = pt.tile([128, GRP * NT], F32, tag="t", name=f"t{nt}_{g}")
                    for j in range(GRP):
                        nc.tensor.matmul(
                            t_ps[:, j * NT:(j + 1) * NT],
                            kt_stack[32 * j:32 * (j + 1), g, :],
                            qt_rep[32 * j:32 * (j + 1), nsl],
                            start=True, stop=True,
                            tile_position=(32 * j, 0),
                        )
                    # e_grp [128, pair(2), half(2), 512] fp8: block 4g+j of
                    # this group lands at [:, j//2, j%2, :] == j-th 512-chunk,
                    # so one ACT exp instruction covers the whole group.
                    # exp split into t-buffer halves: with subtile deps the
                    # next group's first two T-packs only wait on exp_a, so
                    # ACT runs near-continuously instead of serializing the
                    # whole T-pack behind a monolithic exp.
                    e_grp = ep.tile([128, 2, 2, NT], F8, tag="e", name=f"e{nt}_{g}")
                    nc.scalar.activation(
                        out=e_grp[:, 0], in_=t_ps[:, 0:2 * NT], func=FT.Exp
                    )
                    nc.scalar.activation(
                        out=e_grp[:, 1], in_=t_ps[:, 2 * NT:4 * NT], func=FT.Exp
                    )
                    if prev is not None:
                        pending += emit_oz(*prev)
                    if pending:
                        pending.pop(0)()
                    prev = (nt, g, e_grp)
            pending += emit_oz(*prev)
            for p in pending:
                p()

    _split_multiwaits(nc)
    return nc


def _split_multiwaits(nc: bass.Bass) -> None:
    """This container's walrus accepts at most ONE sync-wait per instruction
    (CoreV3GenImpl setupSyncWait). Tile emits multi-wait instructions; split
    the excess waits onto EventSemaphore carriers inserted just before the
    instruction on the same engine — same-engine program order makes this
    semantics-preserving."""
    import json as _json

    data = _json.loads(mybir.module_to_json_bytes(nc.m))
    uid = 0
    for fn in data["functions"]:
        for bb in fn["blocks"]:
            new = []
            for inst in bb["instructions"]:
                si = inst.get("sync_info")
                waits = (si or {}).get("on_wait") or []
                if len(waits) > 1:
                    for wcmd in waits[:-1]:
                        uid += 1
                        new.append({
                            "debug": inst.get("debug", 0),
                            "engine": inst["engine"],
                            "ins": [], "outs": [],
                            "name": f"syncw-{uid}",
                            "opcode": "EventSemaphore",
                            "sync_info": {"on_update": [], "on_wait": [wcmd]},
                        })
                    si["on_wait"] = [waits[-1]]
                new.append(inst)
            bb["instructions"] = new
    nc.m = mybir.module_from_json_bytes(_json.dumps(data).encode())


_NC = None


def _get_nc():
    global _NC
    if _NC is None:
        _NC = build_nc()
    return _NC


def _prep_maps(x, Wf, bf, Wg, bg, Wh, bh, Wo, bo):
    bft = ml_dtypes.bfloat16
    wfg = np.concatenate([Wf, Wg], axis=1)          # [C, 64]
    bfg = np.concatenate([bf, bg], axis=0)          # [64]
    shared = {
        "wfg": np.ascontiguousarray(wfg.reshape(2, 128, 2 * KEY).astype(bft)),
        "wh": np.ascontiguousarray(Wh.reshape(2, 128, C).astype(bft)),
        "wobh": np.ascontiguousarray(Wo.reshape(2, 128, C).astype(bft)),
        "bfgc": np.ascontiguousarray(bfg.reshape(2 * KEY, 1).astype(np.float32)),
        "bhc": np.ascontiguousarray(bh.reshape(2, 128, 1).astype(bft)),
        "bor": np.ascontiguousarray(bo.reshape(1, C).astype(bft)),
    }
    in_maps = []
    for b in range(B):
        xTb = np.ascontiguousarray(
            x[b].reshape(N, C).T.astype(bft).reshape(2, 128, N)
        )
        m = dict(shared)
        m["xT"] = xTb
        in_maps.append(m)
    return in_maps


def run(x, Wf, bf, Wg, bg, Wh, bh, Wo, bo, trace=False, **kw):
    x = np.asarray(x, dtype=np.float32)
    in_maps = _prep_maps(
        x, *(np.asarray(a, dtype=np.float32) for a in (Wf, bf, Wg, bg, Wh, bh, Wo, bo))
    )
    res = run_bass_kernel_spmd(_get_nc(), in_maps, list(range(B)), trace=trace, **kw)
    out = np.empty((B, H, W, C), dtype=np.float32)
    for b in range(B):
        out[b] = np.asarray(res.results[b]["outN"], dtype=np.float32).reshape(H, W, C)
    return out, res


def kernel(x, Wf, bf, Wg, bg, Wh, bh, Wo, bo):
    out, _ = run(x, Wf, bf, Wg, bg, Wh, bh, Wo, bo)
    return out
